# revision 9
# baseline (speedup 1.0000x reference)
"""Trainium2 Bass kernel for scatter-memory GRU update.

reference semantics (single-device jax, CPU):
    current = memory[node_ids]                 # [B, H] gather
    h_new   = GRUCell(messages, current)       # [B, H]
    out     = memory.at[node_ids].set(h_new)   # last occurrence wins

Strategy (8 NeuronCores), bf16 end-to-end (tolerance is 2e-2 rel l2):
  * Shard the 1M-row memory table row-wise: core c owns rows
    [c*125000, (c+1)*125000), split into 4 sub-tables of 31250 rows so
    local row indices fit int16 (dma_gather/dma_scatter_add requirement).
  * Host routes (node_id, message) pairs to the owning (core, sub-table)
    bucket, deduping to the last occurrence per id (matches jax-CPU
    scatter semantics), and casts memory + messages to bf16.
  * Per core: DRAM->DRAM copies its bf16 memory shard to the output,
    dma_gathers the current rows ([item%128, item//128, feat] layout),
    runs the GRU with a 3-stage software pipeline so the in-order PE
    stream [tp_in(g) | matmuls(g-1) | tp_out(g-2)] never waits on the
    same chunk's ACT/DVE chain, and dma_scatter_adds
    delta = h_new - h onto the copied rows (copy + delta == h_new).
  * Padding slots gather row 0 with zero message and scatter into a dummy
    31251st row that the host drops.
  * Host casts the bf16 output back to f32.
"""

import numpy as np

NUM_NODES = 1_000_000
MEM_DIM = 128
N_CORES = 8
N_SUB = 4
ROWS_CORE = NUM_NODES // N_CORES       # 125000
ROWS_SUB = ROWS_CORE // N_SUB          # 31250
N_BUCKETS = N_CORES * N_SUB            # 32
COPY_CHUNK_ROWS = 15625                # 2 chunks of 4 MB per sub-table
CHUNK_ITEMS = 512                      # items per compute chunk
N_GS_CHUNKS = 1                        # gather/scatter calls per sub-table


def _host_prep(node_ids, messages):
    import ml_dtypes
    ids = np.ascontiguousarray(np.asarray(node_ids).astype(np.int64))
    msgs = np.ascontiguousarray(np.asarray(messages).astype(np.float32))
    B = len(ids)
    # unique with LAST occurrence winning (jax-CPU .at[].set semantics)
    u, ri = np.unique(ids[::-1], return_index=True)
    win_pos = B - 1 - ri
    bounds = np.searchsorted(u, np.arange(N_BUCKETS + 1) * ROWS_SUB)
    counts = np.diff(bounds)
    cap = max(512, int(np.ceil(counts.max() / CHUNK_ITEMS) * CHUNK_ITEMS))
    S = cap // 16

    per_core = []
    for c in range(N_CORES):
        gidx = np.zeros((16, N_SUB * S), np.int16)
        sidx = np.zeros((16, N_SUB * S), np.int16)
        msgsT = np.zeros((MEM_DIM, N_SUB * cap), ml_dtypes.bfloat16)
        for k in range(N_SUB):
            b = c * N_SUB + k
            lo, hi = bounds[b], bounds[b + 1]
            n = hi - lo
            loc = (u[lo:hi] - b * ROWS_SUB).astype(np.int16)
            g = np.zeros(cap, np.int16)            # gather pad -> row 0
            s = np.full(cap, ROWS_SUB, np.int16)   # scatter pad -> dummy row
            g[:n] = loc
            s[:n] = loc
            gidx[:, k * S:(k + 1) * S] = g.reshape(S, 16).T
            sidx[:, k * S:(k + 1) * S] = s.reshape(S, 16).T
            msgsT[:, k * cap:k * cap + n] = msgs[win_pos[lo:hi]].T
        per_core.append({
            "gidx": np.ascontiguousarray(np.tile(gidx, (8, 1))),
            "sidx": np.ascontiguousarray(np.tile(sidx, (8, 1))),
            "msgsT": msgsT,
        })
    return per_core, cap


def _build_program(cap, repeats=1, ablate=()):
    import concourse.bass as bass
    import concourse.bacc as bacc
    import concourse.mybir as mybir
    import concourse.tile as tile
    from concourse.masks import make_identity
    from concourse.tile_rust import add_dep_helper

    f32 = mybir.dt.float32
    bf16 = mybir.dt.bfloat16
    i16 = mybir.dt.int16
    AF = mybir.ActivationFunctionType
    S = cap // 16
    n_chunks = cap // CHUNK_ITEMS
    TPC = CHUNK_ITEMS // 128           # 128-blocks per chunk
    total = N_SUB * n_chunks

    nc = bacc.Bacc(None, target_bir_lowering=False)
    mem = [nc.declare_dram_parameter(f"mem{k}", [ROWS_SUB, MEM_DIM], bf16,
                                     isOutput=False) for k in range(N_SUB)]
    msgsT_d = nc.declare_dram_parameter("msgsT", [MEM_DIM, N_SUB * cap], bf16,
                                        isOutput=False)
    wx_d = nc.declare_dram_parameter("wx", [MEM_DIM, 3 * MEM_DIM], bf16,
                                     isOutput=False)
    wh_d = nc.declare_dram_parameter("wh", [MEM_DIM, 3 * MEM_DIM], bf16,
                                     isOutput=False)
    gidx_d = nc.declare_dram_parameter("gidx", [128, N_SUB * S], i16,
                                       isOutput=False)
    sidx_d = nc.declare_dram_parameter("sidx", [128, N_SUB * S], i16,
                                       isOutput=False)
    bias_d = nc.declare_dram_parameter("bias", [MEM_DIM, 4], f32,
                                       isOutput=False)
    out = [nc.declare_dram_parameter(f"out{k}", [ROWS_SUB + 1, MEM_DIM], bf16,
                                     isOutput=True) for k in range(N_SUB)]

    with tile.TileContext(nc) as tc:
        with (
            tc.tile_pool(name="const", bufs=1) as cpool,
            tc.tile_pool(name="hg", bufs=4) as gpool,
            tc.tile_pool(name="hs", bufs=2) as spool,
            tc.tile_pool(name="work", bufs=2) as wpool,
            tc.tile_pool(name="pst", bufs=2, space="PSUM") as ptpool,
            tc.tile_pool(name="psg", bufs=1, space="PSUM") as pgpool,
            tc.tile_pool(name="psd", bufs=2, space="PSUM") as pdpool,
        ):
            wx_sb = cpool.tile([128, 3 * MEM_DIM], bf16)
            wh_sb = cpool.tile([128, 3 * MEM_DIM], bf16)
            b_sb = cpool.tile([128, 4], f32)
            gidx_sb = cpool.tile([128, N_SUB * S], i16)
            sidx_sb = cpool.tile([128, N_SUB * S], i16)
            ident = cpool.tile([128, 128], bf16)
            msgsT_sb = cpool.tile([128, N_SUB * cap], bf16)
            nc.scalar.dma_start(out=wx_sb[:], in_=wx_d[:])
            nc.scalar.dma_start(out=wh_sb[:], in_=wh_d[:])
            nc.scalar.dma_start(out=b_sb[:], in_=bias_d[:])
            nc.scalar.dma_start(out=gidx_sb[:], in_=gidx_d[:])
            nc.scalar.dma_start(out=sidx_sb[:], in_=sidx_d[:])
            nc.scalar.dma_start(out=msgsT_sb[:], in_=msgsT_d[:])
            make_identity(nc, ident[:])

            for rep in range(repeats):
                if rep:
                    tc.strict_bb_all_engine_barrier()

                # ---- stream copies shard -> output (DRAM->DRAM), all up
                # front: they queue FIFO on the SP HWDGE ring and stream in
                # the background while gather/compute run.
                copy_insts = [[] for _ in range(N_SUB)]
                if "copy" not in ablate:
                    for k in range(N_SUB):
                        for j in range(ROWS_SUB // COPY_CHUNK_ROWS):
                            r0 = j * COPY_CHUNK_ROWS
                            r1 = r0 + COPY_CHUNK_ROWS
                            ci = nc.sync.dma_start(out=out[k][r0:r1, :],
                                                   in_=mem[k][r0:r1, :])
                            copy_insts[k].append(ci.ins)

                # ---- gathers, all up front (hg pool bufs=4 so none blocks)
                h_g = []
                for k in range(N_SUB):
                    hgk = gpool.tile([128, cap], bf16, tag="hg")
                    h_g.append(hgk)
                    hg3 = hgk[:].rearrange("p (t d) -> p t d", d=MEM_DIM)
                    gq = cap // N_GS_CHUNKS
                    if "gather" not in ablate:
                        for q in range(N_GS_CHUNKS):
                            nc.gpsimd.dma_gather(
                                out_ap=hg3[:, q * (gq // 128):
                                           (q + 1) * (gq // 128), :],
                                in_ap=mem[k][:, :],
                                idxs_ap=gidx_sb[:, k * S + q * (gq // 16):
                                                k * S + (q + 1) * (gq // 16)],
                                num_idxs=gq,
                                num_idxs_reg=gq,
                                elem_size=MEM_DIM,
                                single_packet=False,
                            )
                    else:
                        nc.gpsimd.memset(hgk[:], 0.0)

                # ---- GRU, 3-stage software pipeline over all chunks ----
                h_sb = []
                for k in range(N_SUB):
                    hsk = spool.tile([128, cap], bf16, tag="hs")
                    h_sb.append(hsk)
                    if "compute" in ablate and "scatter" not in ablate:
                        nc.vector.memset(hsk[:], 0.0)

                stA = {}  # g -> hTc tile
                stB = {}  # g -> dT tile

                def stageA(g):
                    k, c = divmod(g, n_chunks)
                    pt = ptpool.tile([128, CHUNK_ITEMS], bf16, tag="pt",
                                     space="PSUM")
                    for t in range(TPC):
                        nc.tensor.transpose(
                            out=pt[:, t * 128:(t + 1) * 128],
                            in_=h_g[k][:, (c * TPC + t) * 128:
                                       (c * TPC + t + 1) * 128],
                            identity=ident[:])
                    hTc = wpool.tile([128, CHUNK_ITEMS], bf16, tag="hT")
                    nc.scalar.activation(hTc[:], pt[:], AF.Copy)
                    stA[g] = hTc

                def stageB(g):
                    k, c = divmod(g, n_chunks)
                    i0 = c * CHUNK_ITEMS
                    xc = msgsT_sb[:, k * cap + i0:k * cap + i0 + CHUNK_ITEMS]
                    hc = stA.pop(g)[:]

                    psum_r = pgpool.tile([128, CHUNK_ITEMS], f32, tag="pr",
                                         space="PSUM")
                    nc.tensor.matmul(psum_r[:], lhsT=wx_sb[:, 0:128],
                                     rhs=xc, start=True, stop=False)
                    nc.tensor.matmul(psum_r[:], lhsT=wh_sb[:, 0:128],
                                     rhs=hc, start=False, stop=True)
                    psum_z = pgpool.tile([128, CHUNK_ITEMS], f32, tag="pz",
                                         space="PSUM")
                    nc.tensor.matmul(psum_z[:], lhsT=wx_sb[:, 128:256],
                                     rhs=xc, start=True, stop=False)
                    nc.tensor.matmul(psum_z[:], lhsT=wh_sb[:, 128:256],
                                     rhs=hc, start=False, stop=True)
                    psum_gn = pgpool.tile([128, CHUNK_ITEMS], f32, tag="pg",
                                          space="PSUM")
                    nc.tensor.matmul(psum_gn[:], lhsT=wx_sb[:, 256:384],
                                     rhs=xc, start=True, stop=True)
                    psum_hn = pgpool.tile([128, CHUNK_ITEMS], f32, tag="ph",
                                          space="PSUM")
                    nc.tensor.matmul(psum_hn[:], lhsT=wh_sb[:, 256:384],
                                     rhs=hc, start=True, stop=True)

                    r = wpool.tile([128, CHUNK_ITEMS], f32, tag="r")
                    nc.scalar.activation(r[:], psum_r[:], AF.Sigmoid,
                                         bias=b_sb[:, 0:1])
                    zp = wpool.tile([128, CHUNK_ITEMS], f32, tag="zp")
                    nc.scalar.activation(zp[:], psum_z[:], AF.Sigmoid,
                                         bias=b_sb[:, 1:2], scale=-1.0)
                    hnb = wpool.tile([128, CHUNK_ITEMS], f32, tag="hnb")
                    nc.vector.tensor_scalar_add(hnb[:], psum_hn[:],
                                                b_sb[:, 3:4])
                    t1 = wpool.tile([128, CHUNK_ITEMS], f32, tag="t1")
                    nc.vector.tensor_mul(t1[:], r[:], hnb[:])
                    t2 = wpool.tile([128, CHUNK_ITEMS], f32, tag="t2")
                    nc.vector.tensor_add(t2[:], t1[:], psum_gn[:])
                    n_t = wpool.tile([128, CHUNK_ITEMS], f32, tag="nt")
                    nc.scalar.activation(n_t[:], t2[:], AF.Tanh,
                                         bias=b_sb[:, 2:3])
                    nmh = wpool.tile([128, CHUNK_ITEMS], f32, tag="nmh")
                    nc.vector.tensor_sub(nmh[:], n_t[:], hc)
                    dT = wpool.tile([128, CHUNK_ITEMS], bf16, tag="dT")
                    nc.vector.tensor_mul(dT[:], nmh[:], zp[:])
                    stB[g] = dT

                def stageC(g):
                    k, c = divmod(g, n_chunks)
                    i0 = c * CHUNK_ITEMS
                    dT = stB.pop(g)
                    pd = pdpool.tile([128, CHUNK_ITEMS], bf16, tag="pd",
                                     space="PSUM")
                    for t in range(TPC):
                        nc.tensor.transpose(
                            out=pd[:, t * 128:(t + 1) * 128],
                            in_=dT[:, t * 128:(t + 1) * 128],
                            identity=ident[:])
                    nc.vector.tensor_copy(
                        h_sb[k][:, i0:i0 + CHUNK_ITEMS], pd[:])

                def scatter_k(k):
                    hs3 = h_sb[k][:].rearrange("p (t d) -> p t d", d=MEM_DIM)
                    gq = cap // N_GS_CHUNKS
                    for q in range(N_GS_CHUNKS):
                        sc = nc.gpsimd.dma_scatter_add(
                            out[k][:, :],
                            hs3[:, q * (gq // 128):(q + 1) * (gq // 128), :],
                            sidx_sb[:, k * S + q * (gq // 16):
                                    k * S + (q + 1) * (gq // 16)],
                            gq,
                            gq,
                            MEM_DIM,
                            single_packet=False,
                        )
                        for ci in copy_insts[k]:
                            add_dep_helper(sc.ins, ci,
                                           reason="scatter-add after copy")

                if "compute" not in ablate:
                    for g in range(total + 2):
                        if g < total:
                            stageA(g)
                        if 1 <= g < total + 1:
                            stageB(g - 1)
                        if 2 <= g:
                            stageC(g - 2)
                            if (g - 2) % n_chunks == n_chunks - 1 and \
                                    "scatter" not in ablate:
                                scatter_k((g - 2) // n_chunks)
                elif "scatter" not in ablate:
                    for k in range(N_SUB):
                        scatter_k(k)
    nc.compile()
    return nc


def _make_in_maps(inputs, per_core):
    import ml_dtypes
    bfl = ml_dtypes.bfloat16
    memory = np.asarray(inputs["memory"], dtype=np.float32).astype(bfl)
    W_ih = np.asarray(inputs["W_ih"], dtype=np.float32)
    W_hh = np.asarray(inputs["W_hh"], dtype=np.float32)
    b_ih = np.asarray(inputs["b_ih"], dtype=np.float32)
    b_hh = np.asarray(inputs["b_hh"], dtype=np.float32)

    wx = np.ascontiguousarray(W_ih.T).astype(bfl)           # [128, 384]
    wh = np.ascontiguousarray(W_hh.T).astype(bfl)           # [128, 384]
    bias = np.stack([
        b_ih[0:128] + b_hh[0:128],
        -(b_ih[128:256] + b_hh[128:256]),
        b_ih[256:384],
        b_hh[256:384],
    ], axis=1).astype(np.float32)                           # [128, 4]

    in_maps = []
    for c in range(N_CORES):
        m = {
            "msgsT": per_core[c]["msgsT"],
            "gidx": per_core[c]["gidx"],
            "sidx": per_core[c]["sidx"],
            "wx": wx,
            "wh": wh,
            "bias": bias,
        }
        for k in range(N_SUB):
            b = c * N_SUB + k
            m[f"mem{k}"] = np.ascontiguousarray(
                memory[b * ROWS_SUB:(b + 1) * ROWS_SUB])
        in_maps.append(m)
    return in_maps


def _run(inputs, trace=False):
    from concourse.bass_utils import run_bass_kernel_spmd

    per_core, cap = _host_prep(inputs["node_ids"], inputs["messages"])
    in_maps = _make_in_maps(inputs, per_core)
    nc = _build_program(cap)
    res = run_bass_kernel_spmd(nc, in_maps, list(range(N_CORES)),
                               trace=trace)

    outp = np.empty((NUM_NODES, MEM_DIM), np.float32)
    for c in range(N_CORES):
        for k in range(N_SUB):
            b = c * N_SUB + k
            outp[b * ROWS_SUB:(b + 1) * ROWS_SUB] = \
                res.results[c][f"out{k}"][:ROWS_SUB]
    return outp, res


def kernel(**inputs):
    outp, _ = _run(inputs, trace=False)
    return outp
